# revision 1
# baseline (speedup 1.0000x reference)
"""Trainium2 Bass kernel for nn_EntailmentTransformerBlock.

Transformer block: 5-head attention (quirky softmax over the *query* axis),
residual + LN, FFN (640->2560->640), residual + LN.

Sharding: pure data-parallel over batch n (64) across 8 NeuronCores
(8 n-values = 16 (n,s) pairs = 2048 tokens per core).

Device-side layout strategy (per core):
  - Activations flow E-major ("transposed", [E_chunk=128 partitions, tokens])
    through matmuls; token-major [128 tokens, E] for LayerNorms (free-axis
    bn_stats) and DMA.
  - Inputs load token-major (contiguous DMA), PE-transposed to E-major.
  - All matmuls in bf16 with fp32 PSUM accumulation.
  - Quirky softmax(axis=query) is a *free-axis* softmax in the energy^T
    [k_partitions, q_free] layout that the E-major matmul naturally yields.
  - Mask is folded in as a rank-1 (K=1) matmul accumulation of
    ones_k (x) madd_q into the energy PSUM.
"""

import functools

import numpy as np
import ml_dtypes

import concourse.bass as bass
import concourse.tile as tile
from concourse import bacc, mybir
from concourse.bass_utils import run_bass_kernel_spmd
from concourse.masks import make_identity

P = 128
E = 640
EC = 5           # E / 128 chunks
F = 2560
FC = 20          # F / 128 chunks
H = 5            # heads, head_dim = 128
NCORES = 8
NPAIRS = 16      # (n, s) pairs per core: 8 n * 2 s
GROUP = 2        # pairs per processing group
NG = NPAIRS // GROUP
TOKG = GROUP * P  # tokens per group = 256
EPS = 1e-5
SCALE = float(1.0 / np.sqrt(128.0))  # 1/sqrt(key_len)

f32 = mybir.dt.float32
bf16 = mybir.dt.bfloat16

AX = mybir.AxisListType.X
ALU = mybir.AluOpType
ACTF = mybir.ActivationFunctionType


def _bcast_row_ap(ap2d, row):
    """AP reading row `row` of a [R, C] DRAM tensor broadcast over P partitions."""
    row_ap = ap2d[row]
    return bass.AP(
        tensor=row_ap.tensor,
        offset=row_ap.offset,
        ap=[[0, P]] + [list(x) for x in row_ap.ap],
    )


def _layernorm(nc, spool, x1, out, gb, bb, epst):
    """out = ((x1 - mean) * rsqrt(var + eps)) * gb + bb, stats over free axis (640)."""
    st = spool.tile([P, 2, 6], f32, tag="bnst")
    nc.vector.bn_stats(st[:, 0, :], x1[:, 0:320])
    nc.vector.bn_stats(st[:, 1, :], x1[:, 320:640])
    mv = spool.tile([P, 2], f32, tag="mv")
    nc.vector.bn_aggr(mv, st)
    sd = spool.tile([P, 1], f32, tag="sd")
    nc.scalar.activation(sd, mv[:, 1:2], ACTF.Sqrt, bias=epst)  # sqrt(var + eps)
    rstd = spool.tile([P, 1], f32, tag="rstd")
    nc.vector.reciprocal(rstd, sd)
    nmr = spool.tile([P, 1], f32, tag="nmr")
    # nmr = -mean * rstd
    nc.vector.tensor_scalar(nmr, mv[:, 0:1], rstd, -1.0, op0=ALU.mult, op1=ALU.mult)
    # out = x1 * rstd + nmr   (== (x1 - mean) * rstd)
    nc.vector.tensor_scalar(out, x1, rstd, nmr, op0=ALU.mult, op1=ALU.add)
    # gamma / beta on gpsimd (SBUF-only elementwise; keeps DVE free)
    nc.gpsimd.tensor_tensor(out, out, gb, op=ALU.mult)
    nc.gpsimd.tensor_tensor(out, out, bb, op=ALU.add)


def _emit(tc, io, npairs=NPAIRS):
    nc = tc.nc
    ng = npairs // GROUP
    from contextlib import ExitStack

    with ExitStack() as ctx:
        singles = ctx.enter_context(tc.tile_pool(name="singles", bufs=1))
        xpool = ctx.enter_context(tc.tile_pool(name="xall", bufs=npairs))
        ps128 = ctx.enter_context(tc.tile_pool(name="ps128", bufs=2, space="PSUM"))
        ps256 = ctx.enter_context(tc.tile_pool(name="ps256", bufs=2, space="PSUM"))
        psbig = ctx.enter_context(tc.tile_pool(name="psbig", bufs=2, space="PSUM"))

        # ---- constants / weights (resident) ----
        wq_sb = singles.tile([P, EC, E], bf16)
        nc.sync.dma_start(wq_sb, io["wq"].rearrange("(c p) o -> p c o", p=P))
        wk_sb = singles.tile([P, EC, E], bf16)
        nc.sync.dma_start(wk_sb, io["wk"].rearrange("(c p) o -> p c o", p=P))
        wv_sb = singles.tile([P, EC, E], bf16)
        nc.sync.dma_start(wv_sb, io["wv"].rearrange("(c p) o -> p c o", p=P))
        wo_sb = singles.tile([P, EC, E], bf16)
        nc.sync.dma_start(wo_sb, io["wo"].rearrange("(c p) o -> p c o", p=P))
        w1_sb = singles.tile([P, EC, F], bf16)
        nc.scalar.dma_start(w1_sb, io["w1"].rearrange("(c p) o -> p c o", p=P))
        w2_sb = singles.tile([P, FC, E], bf16)
        nc.scalar.dma_start(w2_sb, io["w2"].rearrange("(c p) o -> p c o", p=P))

        bcast = []
        for r in range(6):  # g1, be1, g2, be2, bo, b2
            t = singles.tile([P, E], f32, tag=f"bc{r}")
            nc.gpsimd.dma_start(t, _bcast_row_ap(io["gvecs"], r))
            bcast.append(t)
        g1b, be1b, g2b, be2b, bob, b2b = bcast

        b1t = singles.tile([P, FC], f32)
        nc.scalar.dma_start(b1t, io["b1t"])
        epst = singles.tile([P, 1], f32)
        nc.vector.memset(epst, EPS)
        ident = singles.tile([P, P], f32)
        make_identity(nc, ident)
        ones1 = singles.tile([1, P], bf16)
        nc.vector.memset(ones1, 1.0)

        x_tiles = [None] * npairs  # post-LN1 token-major fp32, per pair

        ncopy = 0

        def copy_eng():
            # alternate PSUM->SBUF copies between DVE and ACT to balance load
            nonlocal ncopy
            ncopy += 1
            return nc.vector if (ncopy % 2) else nc.scalar

        def pcopy(dst, src):
            eng = copy_eng()
            if eng is nc.scalar:
                nc.scalar.copy(dst, src)
            else:
                nc.vector.tensor_copy(dst, src)

        # ================= PASS A: attention + LN1 =================
        with ExitStack() as actx:
            qin_pool = actx.enter_context(tc.tile_pool(name="qin", bufs=4))
            kin_pool = actx.enter_context(tc.tile_pool(name="kin", bufs=2))
            vin_pool = actx.enter_context(tc.tile_pool(name="vin", bufs=2))
            tin_pool = actx.enter_context(tc.tile_pool(name="tin", bufs=2))
            qkt_pool = actx.enter_context(tc.tile_pool(name="qkt", bufs=2))
            vtok_pool = actx.enter_context(tc.tile_pool(name="vtok", bufs=2))
            outt_pool = actx.enter_context(tc.tile_pool(name="outt", bufs=2))
            attn_pool = actx.enter_context(tc.tile_pool(name="attn", bufs=4))
            x1_pool = actx.enter_context(tc.tile_pool(name="x1", bufs=2))
            spool = actx.enter_context(tc.tile_pool(name="stats", bufs=4))
            madd_pool = actx.enter_context(tc.tile_pool(name="madd", bufs=2))

            for g in range(ng):
                # --- load token-major inputs for this group's pairs ---
                qin = []
                kin = []
                vin = []
                for pr in range(GROUP):
                    gp = g * GROUP + pr
                    rows = slice(gp * P, (gp + 1) * P)
                    tq = qin_pool.tile([P, E], f32, tag="qin")
                    nc.sync.dma_start(tq, io["xq"][rows, :])
                    tk = kin_pool.tile([P, E], f32, tag="kin")
                    nc.sync.dma_start(tk, io["xk"][rows, :])
                    tv = vin_pool.tile([P, E], f32, tag="vin")
                    nc.sync.dma_start(tv, io["xv"][rows, :])
                    qin.append(tq)
                    kin.append(tk)
                    vin.append(tv)

                maddt = madd_pool.tile([1, GROUP, P], bf16, tag="madd")
                nc.sync.dma_start(
                    maddt,
                    io["madd"][g * GROUP : (g + 1) * GROUP, :].rearrange(
                        "(o a) b -> o a b", o=1
                    ),
                )

                # --- PE-transpose inputs to E-major bf16 ---
                queryT = tin_pool.tile([P, EC, TOKG], bf16, tag="queryT")
                keyT = tin_pool.tile([P, EC, TOKG], bf16, tag="keyT")
                valueT = tin_pool.tile([P, EC, TOKG], bf16, tag="valueT")
                for src_list, dstT in ((qin, queryT), (kin, keyT), (vin, valueT)):
                    for pr in range(GROUP):
                        for c in range(EC):
                            pst = ps128.tile([P, P], f32, tag="p128")
                            nc.tensor.transpose(
                                pst, src_list[pr][:, c * P : (c + 1) * P], ident
                            )
                            pcopy(dstT[:, c, pr * P : (pr + 1) * P], pst)

                # --- q/k projections (E-major out, all group tokens) ---
                qTb = qkt_pool.tile([P, EC, TOKG], bf16, tag="qTb")
                kTb = qkt_pool.tile([P, EC, TOKG], bf16, tag="kTb")
                for srcT, dst, w_sb in ((queryT, qTb, wq_sb), (keyT, kTb, wk_sb)):
                    for eo in range(EC):
                        ps = ps256.tile([P, TOKG], f32, tag="p256")
                        for ci in range(EC):
                            nc.tensor.matmul(
                                ps,
                                lhsT=w_sb[:, ci, eo * P : (eo + 1) * P],
                                rhs=srcT[:, ci, :],
                                start=(ci == 0),
                                stop=(ci == EC - 1),
                            )
                        pcopy(dst[:, eo, :], ps)

                # --- v projection (token-major out, per pair) ---
                v_tok = vtok_pool.tile([P, GROUP, E], bf16, tag="v_tok")
                for pr in range(GROUP):
                    ps = psbig.tile([P, E], f32, tag="pbig")
                    for n0, nsz in ((0, 512), (512, 128)):
                        for ci in range(EC):
                            nc.tensor.matmul(
                                ps[:, n0 : n0 + nsz],
                                lhsT=valueT[:, ci, pr * P : (pr + 1) * P],
                                rhs=wv_sb[:, ci, n0 : n0 + nsz],
                                start=(ci == 0),
                                stop=(ci == EC - 1),
                            )
                    pcopy(v_tok[:, pr, :], ps)

                # --- attention + output projection + residual + LN1, per pair ---
                outT = outt_pool.tile([P, H, TOKG], bf16, tag="outT")
                x1s = []
                for pr in range(GROUP):
                    gp = g * GROUP + pr
                    tsl = slice(pr * P, (pr + 1) * P)
                    # energy^T for all 5 heads in one 2-bank PSUM tile
                    pse5 = psbig.tile([P, H, P], f32, tag="pbig")
                    for h in range(H):
                        nc.tensor.matmul(
                            pse5[:, h, :], lhsT=kTb[:, h, tsl], rhs=qTb[:, h, tsl],
                            start=True, stop=False,
                        )
                        # + ones_k (x) madd_q  (additive -1e20 on masked q cols)
                        nc.tensor.matmul(
                            pse5[:, h, :], lhsT=ones1, rhs=maddt[:, pr, :],
                            start=False, stop=True,
                        )
                    # batched softmax over q (free axis), scaled by 1/sqrt(128)
                    mx5 = spool.tile([P, H], f32, tag="mx5")
                    nc.vector.reduce_max(out=mx5, in_=pse5, axis=AX)
                    negb5 = spool.tile([P, H], f32, tag="negb5")
                    nc.vector.tensor_scalar_mul(negb5, mx5, -SCALE)
                    attn5 = attn_pool.tile([P, H, P], bf16, tag="asb")
                    # exp emits its own row-sum (softmax denominator) via
                    # accum_out — removes the reduce_sum from the critical chain
                    ssum5 = spool.tile([P, H], f32, tag="ssum5")
                    for h in range(H):
                        nc.scalar.activation(
                            attn5[:, h, :], pse5[:, h, :], ACTF.Exp,
                            bias=negb5[:, h : h + 1], scale=SCALE,
                            accum_out=ssum5[:, h : h + 1],
                        )
                    rec5 = spool.tile([P, H], f32, tag="rec5")
                    nc.vector.reciprocal(rec5, ssum5)
                    nc.vector.tensor_tensor(
                        attn5, attn5, rec5[:, :, None].to_broadcast([P, H, P]),
                        op=ALU.mult,
                    )
                    # out^T[d, h, q] = sum_l v[l,(h,d)] attn^T[h, l, q]
                    pso5 = psbig.tile([P, H, P], f32, tag="pbig")
                    for h in range(H):
                        nc.tensor.matmul(
                            pso5[:, h, :],
                            lhsT=v_tok[:, pr, h * P : (h + 1) * P],
                            rhs=attn5[:, h, :],
                            start=True, stop=True,
                        )
                    pcopy(outT[:, :, tsl], pso5)

                    # attention_out = out @ Wo  (token-major), + bo + query, LN1
                    x1 = x1_pool.tile([P, E], f32, tag="x1")
                    psw = psbig.tile([P, E], f32, tag="pbig")
                    for n0, nsz in ((0, 512), (512, 128)):
                        for h in range(H):
                            nc.tensor.matmul(
                                psw[:, n0 : n0 + nsz],
                                lhsT=outT[:, h, tsl],
                                rhs=wo_sb[:, h, n0 : n0 + nsz],
                                start=(h == 0),
                                stop=(h == H - 1),
                            )
                    nc.vector.tensor_tensor(x1, psw, bob, op=ALU.add)
                    nc.vector.tensor_add(x1, x1, qin[pr])
                    x1s.append((gp, x1))

                # batch the group's LayerNorms so the two ACT Sqrt ops are
                # adjacent — one Exp<->Sqrt table reload per group, not two
                for gp, x1 in x1s:
                    xt = xpool.tile([P, E], f32, tag="xall")
                    _layernorm(nc, spool, x1, xt, g1b, be1b, epst)
                    x_tiles[gp] = xt

        # ================= PASS B: FFN + LN2 =================
        with ExitStack() as bctx:
            xt_pool = bctx.enter_context(tc.tile_pool(name="xT", bufs=2))
            ht_pool = bctx.enter_context(tc.tile_pool(name="hT", bufs=2))
            x2_pool = bctx.enter_context(tc.tile_pool(name="x2", bufs=2))
            out_pool = bctx.enter_context(tc.tile_pool(name="outk", bufs=3))
            spoolb = bctx.enter_context(tc.tile_pool(name="statsb", bufs=4))

            for g in range(ng):
                # transpose x to E-major bf16
                xTb = xt_pool.tile([P, EC, TOKG], bf16, tag="xTb")
                for pr in range(GROUP):
                    gp = g * GROUP + pr
                    for c in range(EC):
                        pst = ps128.tile([P, P], f32, tag="p128")
                        nc.tensor.transpose(
                            pst, x_tiles[gp][:, c * P : (c + 1) * P], ident
                        )
                        pcopy(xTb[:, c, pr * P : (pr + 1) * P], pst)

                # h^T[f, t] = relu(W1^T x^T + b1)
                hT = ht_pool.tile([P, FC, TOKG], bf16, tag="hT")
                for f in range(FC):
                    ps = ps256.tile([P, TOKG], f32, tag="p256")
                    for ci in range(EC):
                        nc.tensor.matmul(
                            ps,
                            lhsT=w1_sb[:, ci, f * P : (f + 1) * P],
                            rhs=xTb[:, ci, :],
                            start=(ci == 0),
                            stop=(ci == EC - 1),
                        )
                    nc.scalar.activation(
                        hT[:, f, :], ps, ACTF.Relu, bias=b1t[:, f : f + 1], scale=1.0
                    )

                # ff = h @ W2 (token-major), + b2 + x, LN2, store
                x2s = []
                for pr in range(GROUP):
                    gp = g * GROUP + pr
                    tsl = slice(pr * P, (pr + 1) * P)
                    x2 = x2_pool.tile([P, E], f32, tag="x2")
                    psf = psbig.tile([P, E], f32, tag="pbig")
                    for n0, nsz in ((0, 512), (512, 128)):
                        for f in range(FC):
                            nc.tensor.matmul(
                                psf[:, n0 : n0 + nsz],
                                lhsT=hT[:, f, tsl],
                                rhs=w2_sb[:, f, n0 : n0 + nsz],
                                start=(f == 0),
                                stop=(f == FC - 1),
                            )
                    nc.vector.tensor_tensor(x2, psf, b2b, op=ALU.add)
                    nc.vector.tensor_add(x2, x2, x_tiles[gp])
                    x2s.append((gp, x2))

                for gp, x2 in x2s:
                    outt = out_pool.tile([P, E], f32, tag="outk")
                    _layernorm(nc, spoolb, x2, outt, g2b, be2b, epst)
                    nc.sync.dma_start(io["out"][gp * P : (gp + 1) * P, :], outt)


@functools.lru_cache(maxsize=4)
def _build(npairs=NPAIRS, repeat=1):
    nc = bacc.Bacc(
        "TRN2", target_bir_lowering=False, debug=False, num_devices=NCORES
    )
    io = {
        "xq": nc.dram_tensor("xq", [npairs * P, E], f32, kind="ExternalInput").ap(),
        "xk": nc.dram_tensor("xk", [npairs * P, E], f32, kind="ExternalInput").ap(),
        "xv": nc.dram_tensor("xv", [npairs * P, E], f32, kind="ExternalInput").ap(),
        "madd": nc.dram_tensor("madd", [npairs, P], bf16, kind="ExternalInput").ap(),
        "wq": nc.dram_tensor("wq", [E, E], bf16, kind="ExternalInput").ap(),
        "wk": nc.dram_tensor("wk", [E, E], bf16, kind="ExternalInput").ap(),
        "wv": nc.dram_tensor("wv", [E, E], bf16, kind="ExternalInput").ap(),
        "wo": nc.dram_tensor("wo", [E, E], bf16, kind="ExternalInput").ap(),
        "w1": nc.dram_tensor("w1", [E, F], bf16, kind="ExternalInput").ap(),
        "w2": nc.dram_tensor("w2", [F, E], bf16, kind="ExternalInput").ap(),
        "b1t": nc.dram_tensor("b1t", [P, FC], f32, kind="ExternalInput").ap(),
        "gvecs": nc.dram_tensor("gvecs", [6, E], f32, kind="ExternalInput").ap(),
        "out": nc.dram_tensor("out", [npairs * P, E], f32, kind="ExternalOutput").ap(),
    }
    with tile.TileContext(nc) as tc:
        for _ in range(repeat):
            _emit(tc, io, npairs)
    nc.compile()
    return nc


def _prep_in_maps(value, key, query, mask, Wv, Wk, Wq, Wo, bo, W1, b1, W2, b2,
                  g1, be1, g2, be2):
    bfl = ml_dtypes.bfloat16
    shared = {
        "wq": np.ascontiguousarray(Wq.astype(bfl)),
        "wk": np.ascontiguousarray(Wk.astype(bfl)),
        "wv": np.ascontiguousarray(Wv.astype(bfl)),
        "wo": np.ascontiguousarray(Wo.astype(bfl)),
        "w1": np.ascontiguousarray(W1.astype(bfl)),
        "w2": np.ascontiguousarray(W2.astype(bfl)),
        "b1t": np.ascontiguousarray(b1.reshape(FC, P).T.astype(np.float32)),
        "gvecs": np.ascontiguousarray(
            np.stack([g1, be1, g2, be2, bo, b2]).astype(np.float32)
        ),
    }
    in_maps = []
    npc = 64 // NCORES  # n-values per core
    for c in range(NCORES):
        nsl = slice(c * npc, (c + 1) * npc)
        madd = np.where(
            mask[nsl, :, :, 0] == 0, np.float32(-1e20), np.float32(0.0)
        ).reshape(NPAIRS, P).astype(bfl)
        in_maps.append(
            {
                # asarray(dtype=...) + ascontiguousarray are no-ops when the
                # slice is already a contiguous f32 view — avoids ~16MB of
                # host memcpy per core on the hot path
                "xq": np.ascontiguousarray(
                    np.asarray(query[nsl].reshape(NPAIRS * P, E), dtype=np.float32)
                ),
                "xk": np.ascontiguousarray(
                    np.asarray(key[nsl].reshape(NPAIRS * P, E), dtype=np.float32)
                ),
                "xv": np.ascontiguousarray(
                    np.asarray(value[nsl].reshape(NPAIRS * P, E), dtype=np.float32)
                ),
                "madd": np.ascontiguousarray(madd),
                **shared,
            }
        )
    return in_maps


def kernel(**inputs) -> np.ndarray:
    nc = _build()
    in_maps = _prep_in_maps(**{
        k: np.asarray(v) for k, v in inputs.items()
    })
    res = run_bass_kernel_spmd(nc, in_maps, core_ids=list(range(NCORES)))
    out = np.concatenate([r["out"] for r in res.results], axis=0)
    return out.reshape(64, 2, P, E).astype(np.float32)


def run_traced(**inputs):
    """Like kernel(), but also returns BassKernelResults with trace info."""
    nc = _build()
    in_maps = _prep_in_maps(**{k: np.asarray(v) for k, v in inputs.items()})
    res = run_bass_kernel_spmd(
        nc, in_maps, core_ids=list(range(NCORES)), trace=True
    )
    out = np.concatenate([r["out"] for r in res.results], axis=0)
    return out.reshape(64, 2, P, E).astype(np.float32), res



# revision 18
# speedup vs baseline: 81.3287x; 81.3287x over previous
"""Trainium2 Bass kernel for nn_EntailmentTransformerBlock.

Transformer block: 5-head attention (quirky softmax over the *query* axis),
residual + LN, FFN (640->2560->640), residual + LN.

Sharding: pure data-parallel over batch n (64) across 8 NeuronCores
(8 n-values = 16 (n,s) pairs = 2048 tokens per core).

v3 design notes (PE-sequencer/issue-bound -> minimize PE instruction count
and keep PE continuously fed):
  - GROUP=4 pairs per step -> 512-token matmuls (max f32 PSUM width).
  - q/k/v E-major tiles come straight from DRAM via DMA xbar transposes
    (bf16, host-cast); no PE transposes, no token-major staging for k/v.
  - Mask folded in as a DVE add of a partition-broadcast (-1e20) tile onto
    the energy PSUM (softmax over the query axis = free axis).
  - Softmax without max-subtraction (energies ~ N(0,1) after 1/sqrt(L)):
    one Exp per pair over all 5 heads, denominators via one DVE reduce.
  - LN1 normalization batched per group: one ACT Sqrt per group keeps the
    Exp<->Sqrt table swaps to 2 per group instead of 2 per pair.
  - LN1 affine folded into W1/b1 on host (W1' = g1*W1, b1' = b1 + be1@W1).
  - FFN interleaved f-outer: each relu'd hT chunk feeds FFN2 partial
    accumulation into 5 live PSUM tiles immediately -> no hT megabuffer,
    no FFN1->FFN2 serialization.
  - FFN2 is E-major (weights as lhsT, 512-wide); bias b2+be1 added in the
    PSUM->SBUF copy; DMA-transposed back for residual + LN2.
  - W1 prefetches during pass A (top-level pool); attention weights live
    only in pass A, W2 + LN vectors only in pass B.
"""

import functools

import numpy as np
import ml_dtypes

import concourse.bass as bass
import concourse.tile as tile
from concourse import bacc, mybir
from concourse.bass_utils import run_bass_kernel_spmd

P = 128
E = 640
EC = 5           # E / 128 chunks
F = 2560
FC = 20          # F / 128 chunks
H = 5            # heads, head_dim = 128
NCORES = 8
NPAIRS = 16      # (n, s) pairs per core: 8 n * 2 s
GROUP = 4        # pairs per processing group
NG = NPAIRS // GROUP
TOKG = GROUP * P  # tokens per group = 512
EPS = 1e-5
SCALE = float(1.0 / np.sqrt(128.0))  # 1/sqrt(key_len)

f32 = mybir.dt.float32
bf16 = mybir.dt.bfloat16

AX = mybir.AxisListType.X
ALU = mybir.AluOpType
ACTF = mybir.ActivationFunctionType


def _bcast_rows_ap(ap2d, r0, nrows):
    """AP reading rows [r0, r0+nrows) of a [R, C] DRAM tensor, each row
    broadcast over P partitions: result is [P, nrows, C]."""
    row_ap = ap2d[r0]
    inner = [list(x) for x in row_ap.ap]  # [[1, C]]
    row_stride = ap2d.ap[-2][0]
    return bass.AP(
        tensor=row_ap.tensor,
        offset=row_ap.offset,
        ap=[[0, P], [row_stride, nrows]] + inner,
    )


def _bcast_row_ap(ap2d, row):
    row_ap = ap2d[row]
    return bass.AP(
        tensor=row_ap.tensor,
        offset=row_ap.offset,
        ap=[[0, P]] + [list(x) for x in row_ap.ap],
    )


def _emit(tc, io, npairs=NPAIRS):
    nc = tc.nc
    ng = npairs // GROUP
    from contextlib import ExitStack

    with ExitStack() as ctx:
        singles = ctx.enter_context(tc.tile_pool(name="singles", bufs=1))
        xnpool = ctx.enter_context(tc.tile_pool(name="xnb", bufs=ng))

        epst = singles.tile([P, 1], f32)
        nc.vector.memset(epst, EPS)
        # W1/W2 prefetch tiles: DMAs are issued in pass A (after the
        # attention weights, so they don't block the first projections)
        w1_sb = singles.tile([P, EC, F], bf16, tag="w1")
        w2_sb = singles.tile([P, FC, E], bf16, tag="w2")

        ncopy = 0

        def pcopy(dst, src):
            # alternate PSUM->SBUF copies between DVE and ACT
            nonlocal ncopy
            ncopy += 1
            if ncopy % 2:
                nc.vector.tensor_copy(dst, src)
            else:
                nc.scalar.activation(dst, src, ACTF.Identity)

        xnb_tiles = [None] * ng    # post-LN1 normalized bf16, [P, GROUP, E]

        # ================= PASS A: attention =================
        with ExitStack() as actx:
            wpool = actx.enter_context(tc.tile_pool(name="wA", bufs=1))
            tin_pool = actx.enter_context(tc.tile_pool(name="tin", bufs=2))
            tT_pool = actx.enter_context(tc.tile_pool(name="tT", bufs=2))
            qkt_pool = actx.enter_context(tc.tile_pool(name="qkt", bufs=2))
            vtok_pool = actx.enter_context(tc.tile_pool(name="vtok", bufs=2))
            outt_pool = actx.enter_context(tc.tile_pool(name="outt", bufs=2))
            attn_pool = actx.enter_context(tc.tile_pool(name="attn", bufs=4))
            spool = actx.enter_context(tc.tile_pool(name="stats", bufs=4))
            madd_pool = actx.enter_context(tc.tile_pool(name="madd", bufs=2))
            x1_pool = actx.enter_context(tc.tile_pool(name="x1", bufs=6))
            psA = actx.enter_context(tc.tile_pool(name="psA", bufs=2, space="PSUM"))
            psbig = actx.enter_context(tc.tile_pool(name="psbig", bufs=3, space="PSUM"))

            wq_sb = wpool.tile([P, EC, E], bf16, tag="wq")
            nc.gpsimd.dma_start(wq_sb, io["wq"].rearrange("(c p) o -> p c o", p=P))
            wk_sb = wpool.tile([P, EC, E], bf16, tag="wk")
            nc.gpsimd.dma_start(wk_sb, io["wk"].rearrange("(c p) o -> p c o", p=P))
            wv_sb = wpool.tile([P, EC, E], bf16, tag="wv")
            nc.gpsimd.dma_start(wv_sb, io["wv"].rearrange("(c p) o -> p c o", p=P))
            wo_sb = wpool.tile([P, EC, E], bf16, tag="wo")
            nc.gpsimd.dma_start(wo_sb, io["wo"].rearrange("(c p) o -> p c o", p=P))
            bob = wpool.tile([P, E], f32, tag="bob")
            nc.gpsimd.dma_start(bob, _bcast_row_ap(io["gvecs"], 3))

            for g in range(ng):
                # FFN weight prefetch: issued once group-0 inputs are in
                # flight so it overlaps pass A compute instead of blocking
                # the first projections
                if g == 1:
                    nc.gpsimd.dma_start(
                        w1_sb, io["w1"].rearrange("(c p) o -> p c o", p=P)
                    )
                elif g == 2:
                    nc.gpsimd.dma_start(
                        w2_sb, io["w2"].rearrange("(c p) o -> p c o", p=P)
                    )
                rows = slice(g * TOKG, (g + 1) * TOKG)
                # token-major q (residual only)
                qb = tin_pool.tile([P, GROUP, E], bf16, tag="qb")
                nc.sync.dma_start(
                    qb, io["xq"][rows, :].rearrange("(a p) e -> p a e", p=P)
                )
                # E-major q/k/v straight from DRAM via xbar transpose
                qT = tT_pool.tile([P, EC, TOKG], bf16, tag="qT")
                nc.sync.dma_start_transpose(qT, io["xq"][rows, :])
                kT = tT_pool.tile([P, EC, TOKG], bf16, tag="kT")
                nc.sync.dma_start_transpose(kT, io["xk"][rows, :])
                vT = tT_pool.tile([P, EC, TOKG], bf16, tag="vT")
                nc.scalar.dma_start_transpose(vT, io["xv"][rows, :])

                # mask additive tile, broadcast across partitions
                maddt = madd_pool.tile([P, GROUP, P], f32, tag="madd")
                nc.gpsimd.dma_start(
                    maddt, _bcast_rows_ap(io["madd"], g * GROUP, GROUP)
                )

                # --- q/k projections, E-major [P, H, TOKG] ---
                qTb = qkt_pool.tile([P, H, TOKG], bf16, tag="qTb")
                kTb = qkt_pool.tile([P, H, TOKG], bf16, tag="kTb")
                for srcT, dst, w_sb in ((qT, qTb, wq_sb), (kT, kTb, wk_sb)):
                    for eo in range(EC):
                        ps = psA.tile([P, TOKG], f32, tag="pA")
                        for ci in range(EC):
                            nc.tensor.matmul(
                                ps,
                                lhsT=w_sb[:, ci, eo * P : (eo + 1) * P],
                                rhs=srcT[:, ci, :],
                                start=(ci == 0),
                                stop=(ci == EC - 1),
                            )
                        pcopy(dst[:, eo, :], ps)

                # --- v projection (token-major out, per pair; ci outer) ---
                v_tok = vtok_pool.tile([P, GROUP, E], bf16, tag="v_tok")
                for a in range(GROUP):
                    pv = psbig.tile([P, E], f32, tag="pbig")
                    for ci in range(EC):
                        for n0, nsz in ((0, 512), (512, 128)):
                            nc.tensor.matmul(
                                pv[:, n0 : n0 + nsz],
                                lhsT=vT[:, ci, a * P : (a + 1) * P],
                                rhs=wv_sb[:, ci, n0 : n0 + nsz],
                                start=(ci == 0),
                                stop=(ci == EC - 1),
                            )
                    pcopy(v_tok[:, a, :], pv)

                # --- attention + out-proj + residual, in 2-pair waves so
                # pair a+1's PE work hides pair a's softmax latency ---
                outT = outt_pool.tile([P, H, TOKG], bf16, tag="outT")
                mv4 = spool.tile([P, GROUP, 2], f32, tag="mv4")
                x1s = [None] * GROUP
                attn5s = [None] * GROUP

                def _energy_softmax(a):
                    tsl = slice(a * P, (a + 1) * P)
                    # energy^T[k, (h, q)] for all 5 heads
                    pse5 = psbig.tile([P, H, P], f32, tag="pbig")
                    for h in range(H):
                        nc.tensor.matmul(
                            pse5[:, h, :], lhsT=kTb[:, h, tsl], rhs=qTb[:, h, tsl],
                            start=True, stop=True,
                        )
                    # + additive mask on the q (free) axis
                    nc.vector.tensor_tensor(
                        pse5, pse5,
                        maddt[:, a, None, :].to_broadcast([P, H, P]),
                        op=ALU.add,
                    )
                    # softmax over q (free axis), scaled by 1/sqrt(128);
                    # energies are ~N(0,1) after scaling -> no max-sub needed
                    attn5 = attn_pool.tile([P, H, P], bf16, tag="asb")
                    nc.scalar.activation(attn5, pse5, ACTF.Exp, scale=SCALE)
                    ssum5 = spool.tile([P, H], f32, tag="ssum5")
                    nc.vector.tensor_reduce(ssum5, attn5, op=ALU.add, axis=AX)
                    rec5 = spool.tile([P, H], f32, tag="rec5")
                    nc.vector.reciprocal(rec5, ssum5)
                    nc.vector.tensor_tensor(
                        attn5, attn5, rec5[:, :, None].to_broadcast([P, H, P]),
                        op=ALU.mult,
                    )
                    attn5s[a] = attn5

                def _av(a):
                    tsl = slice(a * P, (a + 1) * P)
                    # out^T[d, h, q] = sum_l v[l,(h,d)] attn^T[h, l, q]
                    pso5 = psbig.tile([P, H, P], f32, tag="pbig")
                    for h in range(H):
                        nc.tensor.matmul(
                            pso5[:, h, :],
                            lhsT=v_tok[:, a, h * P : (h + 1) * P],
                            rhs=attn5s[a][:, h, :],
                            start=True, stop=True,
                        )
                    pcopy(outT[:, :, tsl], pso5)

                def _woproj(a):
                    tsl = slice(a * P, (a + 1) * P)
                    # attention_out = out @ Wo (token-major; h outer to share
                    # weights across the two PSUM column regions)
                    psw = psbig.tile([P, E], f32, tag="pbig")
                    for h in range(H):
                        for n0, nsz in ((0, 512), (512, 128)):
                            nc.tensor.matmul(
                                psw[:, n0 : n0 + nsz],
                                lhsT=outT[:, h, tsl],
                                rhs=wo_sb[:, h, n0 : n0 + nsz],
                                start=(h == 0),
                                stop=(h == H - 1),
                            )
                    # x1 = psw + bo + query (bf16 residual), stats
                    x1 = x1_pool.tile([P, E], f32, tag="x1")
                    nc.vector.tensor_tensor(x1, psw, bob, op=ALU.add)
                    nc.vector.tensor_add(x1, x1, qb[:, a, :])
                    st = spool.tile([P, 2, 6], f32, tag="bnst")
                    nc.vector.bn_stats(st[:, 0, :], x1[:, 0:320])
                    nc.vector.bn_stats(st[:, 1, :], x1[:, 320:640])
                    nc.vector.bn_aggr(mv4[:, a, :], st)
                    x1s[a] = x1

                for w in (0, 2):
                    _energy_softmax(w)
                    _energy_softmax(w + 1)
                    _av(w)
                    _av(w + 1)
                    _woproj(w)
                    _woproj(w + 1)

                # batched LN1 normalize: one ACT Sqrt per group (2 table
                # swaps per group instead of 2 per pair)
                sd4 = spool.tile([P, GROUP], f32, tag="sd4")
                nc.scalar.activation(sd4, mv4[:, :, 1], ACTF.Sqrt, bias=epst)
                rstd4 = spool.tile([P, GROUP], f32, tag="rstd4")
                nc.vector.reciprocal(rstd4, sd4)
                nmr4 = spool.tile([P, GROUP], f32, tag="nmr4")
                nc.vector.tensor_tensor(nmr4, mv4[:, :, 0], rstd4, op=ALU.mult)
                nc.vector.tensor_scalar_mul(nmr4, nmr4, -1.0)
                xnb = xnpool.tile([P, GROUP, E], bf16, tag="xnb")
                for a in range(GROUP):
                    nc.vector.tensor_scalar(
                        xnb[:, a, :], x1s[a],
                        rstd4[:, a : a + 1], nmr4[:, a : a + 1],
                        op0=ALU.mult, op1=ALU.add,
                    )
                xnb_tiles[g] = xnb

        # ================= PASS B: FFN + LN2 =================
        with ExitStack() as bctx:
            wbpool = bctx.enter_context(tc.tile_pool(name="wB", bufs=1))
            xt_pool = bctx.enter_context(tc.tile_pool(name="xT", bufs=2))
            ht_pool = bctx.enter_context(tc.tile_pool(name="hT", bufs=4))
            fft_pool = bctx.enter_context(tc.tile_pool(name="ffT", bufs=2))
            fftok_pool = bctx.enter_context(tc.tile_pool(name="fftok", bufs=2))
            x2_pool = bctx.enter_context(tc.tile_pool(name="x2", bufs=4))
            out_pool = bctx.enter_context(tc.tile_pool(name="outg", bufs=4))
            spoolb = bctx.enter_context(tc.tile_pool(name="statsb", bufs=4))
            psB = bctx.enter_context(tc.tile_pool(name="psB", bufs=2, space="PSUM"))
            psf_pool = bctx.enter_context(
                tc.tile_pool(name="psf", bufs=EC, space="PSUM")
            )

            bvec = []
            for r in range(3):  # g1, g2, be2
                t = wbpool.tile([P, E], f32, tag=f"bc{r}")
                nc.gpsimd.dma_start(t, _bcast_row_ap(io["gvecs"], r))
                bvec.append(t)
            g1b, g2b, be2b = bvec
            b1t = wbpool.tile([P, FC], f32, tag="b1t")
            nc.gpsimd.dma_start(b1t, io["b1t"])
            b2t = wbpool.tile([P, EC], f32, tag="b2t")
            nc.gpsimd.dma_start(b2t, io["b2t"])

            for g in range(ng):
                # E-major x via DMA xbar transpose: [P, (a, eo), t]
                xT = xt_pool.tile([P, GROUP, EC, P], bf16, tag="xT")
                nc.sync.dma_start_transpose(xT, xnb_tiles[g])

                # interleaved FFN: each hT chunk immediately feeds the 5
                # E-major FFN2 partial accumulators
                psf = []
                for eo in range(EC):
                    pf = psf_pool.tile([P, TOKG], f32, tag="psf")
                    psf.append(pf)
                for f in range(FC):
                    ps = psB.tile([P, TOKG], f32, tag="pB")
                    for ci in range(EC):
                        nc.tensor.matmul(
                            ps,
                            lhsT=w1_sb[:, ci, f * P : (f + 1) * P],
                            rhs=xT[:, :, ci, :],
                            start=(ci == 0),
                            stop=(ci == EC - 1),
                        )
                    ht = ht_pool.tile([P, TOKG], bf16, tag="ht")
                    if f % 4 == 3:
                        # DVE takes every 4th chunk; ACT (which has slack)
                        # takes the rest so DVE stays free for LN2 chains
                        nc.vector.tensor_scalar(
                            ht, ps, b1t[:, f : f + 1], 0.0,
                            op0=ALU.add, op1=ALU.max,
                        )
                    else:
                        nc.scalar.activation(
                            ht, ps, ACTF.Relu, bias=b1t[:, f : f + 1], scale=1.0
                        )
                    for eo in range(EC):
                        nc.tensor.matmul(
                            psf[eo],
                            lhsT=w2_sb[:, f, eo * P : (eo + 1) * P],
                            rhs=ht,
                            start=(f == 0),
                            stop=(f == FC - 1),
                        )

                # ff^T + (b2 + be1) -> bf16, [P, (a, eo), t] layout
                ffT = fft_pool.tile([P, GROUP, EC, P], bf16, tag="ffT")
                for eo in range(EC):
                    src = psf[eo].rearrange("p (a t) -> p a t", a=GROUP)
                    # all on ACT: frees psf slots for the next group's FFN2
                    # without waiting on DVE (busy with LN2 chains)
                    nc.scalar.activation(
                        ffT[:, :, eo, :], src, ACTF.Identity,
                        bias=b2t[:, eo : eo + 1],
                    )

                # back to token-major per pair
                ff_tok = fftok_pool.tile([P, GROUP, E], bf16, tag="fftok")
                for a in range(GROUP):
                    # out must stay 3D [P, EC, P]: a 2D [P, 640] view makes
                    # dma_start_transpose mis-tile the xbar output
                    nc.scalar.dma_start_transpose(
                        ff_tok[:, a].rearrange("p (c t) -> p c t", c=EC),
                        ffT[:, a],
                    )

                # x2 = xnb * g1 + ff (be1 + b2 already in ff), then LN2
                for a in range(GROUP):
                    x2 = x2_pool.tile([P, E], f32, tag="x2")
                    nc.gpsimd.tensor_tensor(
                        x2, xnb_tiles[g][:, a, :], g1b, op=ALU.mult
                    )
                    nc.vector.tensor_add(x2, x2, ff_tok[:, a, :])
                    st = spoolb.tile([P, 2, 6], f32, tag="bnstb")
                    nc.vector.bn_stats(st[:, 0, :], x2[:, 0:320])
                    nc.vector.bn_stats(st[:, 1, :], x2[:, 320:640])
                    mv = spoolb.tile([P, 2], f32, tag="mvb")
                    nc.vector.bn_aggr(mv, st)
                    sd = spoolb.tile([P, 1], f32, tag="sdb")
                    nc.scalar.activation(sd, mv[:, 1:2], ACTF.Sqrt, bias=epst)
                    rstd = spoolb.tile([P, 1], f32, tag="rstdb")
                    nc.vector.reciprocal(rstd, sd)
                    nmr = spoolb.tile([P, 1], f32, tag="nmrb")
                    nc.vector.tensor_scalar(
                        nmr, mv[:, 0:1], rstd, -1.0, op0=ALU.mult, op1=ALU.mult
                    )
                    xo = x2_pool.tile([P, E], f32, tag="xo")
                    nc.vector.tensor_scalar(
                        xo, x2, rstd, nmr, op0=ALU.mult, op1=ALU.add
                    )
                    # affine split across Pool/DVE, alternating per pair;
                    # store per pair so the tail drains promptly
                    t2 = x2_pool.tile([P, E], f32, tag="t2")
                    outp = out_pool.tile([P, E], f32, tag="outp")
                    if a % 2:
                        nc.gpsimd.tensor_tensor(t2, xo, g2b, op=ALU.mult)
                        nc.vector.tensor_add(outp, t2, be2b)
                    else:
                        nc.vector.tensor_tensor(t2, xo, g2b, op=ALU.mult)
                        nc.gpsimd.tensor_tensor(outp, t2, be2b, op=ALU.add)
                    nc.sync.dma_start(
                        io["out"][
                            g * TOKG + a * P : g * TOKG + (a + 1) * P, :
                        ],
                        outp,
                    )


@functools.lru_cache(maxsize=8)
def _build(npairs=NPAIRS, repeat=1, loop=1):
    nc = bacc.Bacc(
        "TRN2", target_bir_lowering=False, debug=False, num_devices=NCORES
    )
    io = {
        "xq": nc.dram_tensor("xq", [npairs * P, E], bf16, kind="ExternalInput").ap(),
        "xk": nc.dram_tensor("xk", [npairs * P, E], bf16, kind="ExternalInput").ap(),
        "xv": nc.dram_tensor("xv", [npairs * P, E], bf16, kind="ExternalInput").ap(),
        "madd": nc.dram_tensor("madd", [npairs, P], f32, kind="ExternalInput").ap(),
        "wq": nc.dram_tensor("wq", [E, E], bf16, kind="ExternalInput").ap(),
        "wk": nc.dram_tensor("wk", [E, E], bf16, kind="ExternalInput").ap(),
        "wv": nc.dram_tensor("wv", [E, E], bf16, kind="ExternalInput").ap(),
        "wo": nc.dram_tensor("wo", [E, E], bf16, kind="ExternalInput").ap(),
        "w1": nc.dram_tensor("w1", [E, F], bf16, kind="ExternalInput").ap(),
        "w2": nc.dram_tensor("w2", [F, E], bf16, kind="ExternalInput").ap(),
        "b1t": nc.dram_tensor("b1t", [P, FC], f32, kind="ExternalInput").ap(),
        "b2t": nc.dram_tensor("b2t", [P, EC], f32, kind="ExternalInput").ap(),
        "gvecs": nc.dram_tensor("gvecs", [4, E], f32, kind="ExternalInput").ap(),
        "out": nc.dram_tensor("out", [npairs * P, E], f32, kind="ExternalOutput").ap(),
    }
    with tile.TileContext(nc) as tc:
        if loop > 1:
            # hardware loop: R full iterations inside one launch (for timing)
            with tc.For_i(0, loop, 1):
                for _ in range(repeat):
                    _emit(tc, io, npairs)
        else:
            for _ in range(repeat):
                _emit(tc, io, npairs)
    nc.compile()
    return nc


def _prep_in_maps(value, key, query, mask, Wv, Wk, Wq, Wo, bo, W1, b1, W2, b2,
                  g1, be1, g2, be2):
    bfl = ml_dtypes.bfloat16
    w1p = (np.asarray(g1, np.float32)[:, None] * np.asarray(W1, np.float32))
    b1p = np.asarray(b1, np.float32) + np.asarray(be1, np.float32) @ np.asarray(
        W1, np.float32
    )
    b2p = np.asarray(b2, np.float32) + np.asarray(be1, np.float32)
    shared = {
        "wq": np.ascontiguousarray(Wq.astype(bfl)),
        "wk": np.ascontiguousarray(Wk.astype(bfl)),
        "wv": np.ascontiguousarray(Wv.astype(bfl)),
        "wo": np.ascontiguousarray(Wo.astype(bfl)),
        "w1": np.ascontiguousarray(w1p.astype(bfl)),
        "w2": np.ascontiguousarray(W2.astype(bfl)),
        "b1t": np.ascontiguousarray(b1p.reshape(FC, P).T.astype(np.float32)),
        "b2t": np.ascontiguousarray(b2p.reshape(EC, P).T.astype(np.float32)),
        "gvecs": np.ascontiguousarray(
            np.stack([g1, g2, be2, bo]).astype(np.float32)
        ),
    }
    in_maps = []
    npc = 64 // NCORES  # n-values per core
    for c in range(NCORES):
        nsl = slice(c * npc, (c + 1) * npc)
        madd = np.where(
            mask[nsl, :, :, 0] == 0, np.float32(-1e20), np.float32(0.0)
        ).reshape(NPAIRS, P).astype(np.float32)
        in_maps.append(
            {
                "xq": np.ascontiguousarray(
                    query[nsl].reshape(NPAIRS * P, E).astype(bfl)
                ),
                "xk": np.ascontiguousarray(
                    key[nsl].reshape(NPAIRS * P, E).astype(bfl)
                ),
                "xv": np.ascontiguousarray(
                    value[nsl].reshape(NPAIRS * P, E).astype(bfl)
                ),
                "madd": np.ascontiguousarray(madd),
                **shared,
            }
        )
    return in_maps


def kernel(**inputs) -> np.ndarray:
    nc = _build()
    in_maps = _prep_in_maps(**{
        k: np.asarray(v) for k, v in inputs.items()
    })
    res = run_bass_kernel_spmd(nc, in_maps, core_ids=list(range(NCORES)))
    out = np.concatenate([r["out"] for r in res.results], axis=0)
    return out.reshape(64, 2, P, E).astype(np.float32)


def run_traced(**inputs):
    """Like kernel(), but also returns BassKernelResults with trace info."""
    nc = _build()
    in_maps = _prep_in_maps(**{k: np.asarray(v) for k, v in inputs.items()})
    res = run_bass_kernel_spmd(
        nc, in_maps, core_ids=list(range(NCORES)), trace=True
    )
    out = np.concatenate([r["out"] for r in res.results], axis=0)
    return out.reshape(64, 2, P, E).astype(np.float32), res


# revision 19
# speedup vs baseline: 82.6347x; 1.0161x over previous
"""Trainium2 Bass kernel for nn_EntailmentTransformerBlock.

Transformer block: 5-head attention (quirky softmax over the *query* axis),
residual + LN, FFN (640->2560->640), residual + LN.

Sharding: pure data-parallel over batch n (64) across 8 NeuronCores
(8 n-values = 16 (n,s) pairs = 2048 tokens per core).

v3 design notes (PE-sequencer/issue-bound -> minimize PE instruction count
and keep PE continuously fed):
  - GROUP=4 pairs per step -> 512-token matmuls (max f32 PSUM width).
  - q/k/v E-major tiles come straight from DRAM via DMA xbar transposes
    (bf16, host-cast); no PE transposes, no token-major staging for k/v.
  - Mask folded in as a DVE add of a partition-broadcast (-1e20) tile onto
    the energy PSUM (softmax over the query axis = free axis).
  - Softmax without max-subtraction (energies ~ N(0,1) after 1/sqrt(L)):
    one Exp per pair over all 5 heads, denominators via one DVE reduce.
  - LN1 normalization batched per group: one ACT Sqrt per group keeps the
    Exp<->Sqrt table swaps to 2 per group instead of 2 per pair.
  - LN1 affine folded into W1/b1 on host (W1' = g1*W1, b1' = b1 + be1@W1).
  - FFN interleaved f-outer: each relu'd hT chunk feeds FFN2 partial
    accumulation into 5 live PSUM tiles immediately -> no hT megabuffer,
    no FFN1->FFN2 serialization.
  - FFN2 is E-major (weights as lhsT, 512-wide); bias b2+be1 added in the
    PSUM->SBUF copy; DMA-transposed back for residual + LN2.
  - W1 prefetches during pass A (top-level pool); attention weights live
    only in pass A, W2 + LN vectors only in pass B.
"""

import functools

import numpy as np
import ml_dtypes

import concourse.bass as bass
import concourse.tile as tile
from concourse import bacc, mybir
from concourse.bass_utils import run_bass_kernel_spmd

P = 128
E = 640
EC = 5           # E / 128 chunks
F = 2560
FC = 20          # F / 128 chunks
H = 5            # heads, head_dim = 128
NCORES = 8
NPAIRS = 16      # (n, s) pairs per core: 8 n * 2 s
GROUP = 4        # pairs per processing group
NG = NPAIRS // GROUP
TOKG = GROUP * P  # tokens per group = 512
EPS = 1e-5
SCALE = float(1.0 / np.sqrt(128.0))  # 1/sqrt(key_len)

f32 = mybir.dt.float32
bf16 = mybir.dt.bfloat16

AX = mybir.AxisListType.X
ALU = mybir.AluOpType
ACTF = mybir.ActivationFunctionType


def _bcast_rows_ap(ap2d, r0, nrows):
    """AP reading rows [r0, r0+nrows) of a [R, C] DRAM tensor, each row
    broadcast over P partitions: result is [P, nrows, C]."""
    row_ap = ap2d[r0]
    inner = [list(x) for x in row_ap.ap]  # [[1, C]]
    row_stride = ap2d.ap[-2][0]
    return bass.AP(
        tensor=row_ap.tensor,
        offset=row_ap.offset,
        ap=[[0, P], [row_stride, nrows]] + inner,
    )


def _bcast_row_ap(ap2d, row):
    row_ap = ap2d[row]
    return bass.AP(
        tensor=row_ap.tensor,
        offset=row_ap.offset,
        ap=[[0, P]] + [list(x) for x in row_ap.ap],
    )


def _emit(tc, io, npairs=NPAIRS):
    nc = tc.nc
    ng = npairs // GROUP
    from contextlib import ExitStack

    with ExitStack() as ctx:
        singles = ctx.enter_context(tc.tile_pool(name="singles", bufs=1))
        xnpool = ctx.enter_context(tc.tile_pool(name="xnb", bufs=ng))

        epst = singles.tile([P, 1], f32)
        nc.vector.memset(epst, EPS)
        # W1/W2 prefetch tiles: DMAs are issued in pass A (after the
        # attention weights, so they don't block the first projections)
        w1_sb = singles.tile([P, EC, F], bf16, tag="w1")
        w2_sb = singles.tile([P, FC, E], bf16, tag="w2")

        ncopy = 0

        def pcopy(dst, src):
            # alternate PSUM->SBUF copies between DVE and ACT
            nonlocal ncopy
            ncopy += 1
            if ncopy % 2:
                nc.vector.tensor_copy(dst, src)
            else:
                nc.scalar.activation(dst, src, ACTF.Identity)

        xnb_tiles = [None] * ng    # post-LN1 normalized bf16, [P, GROUP, E]

        # ================= PASS A: attention =================
        with ExitStack() as actx:
            wpool = actx.enter_context(tc.tile_pool(name="wA", bufs=1))
            tin_pool = actx.enter_context(tc.tile_pool(name="tin", bufs=2))
            tT_pool = actx.enter_context(tc.tile_pool(name="tT", bufs=2))
            qkt_pool = actx.enter_context(tc.tile_pool(name="qkt", bufs=2))
            vtok_pool = actx.enter_context(tc.tile_pool(name="vtok", bufs=2))
            outt_pool = actx.enter_context(tc.tile_pool(name="outt", bufs=2))
            attn_pool = actx.enter_context(tc.tile_pool(name="attn", bufs=4))
            spool = actx.enter_context(tc.tile_pool(name="stats", bufs=4))
            madd_pool = actx.enter_context(tc.tile_pool(name="madd", bufs=2))
            x1_pool = actx.enter_context(tc.tile_pool(name="x1", bufs=6))
            psA = actx.enter_context(tc.tile_pool(name="psA", bufs=2, space="PSUM"))
            psbig = actx.enter_context(tc.tile_pool(name="psbig", bufs=3, space="PSUM"))

            wq_sb = wpool.tile([P, EC, E], bf16, tag="wq")
            nc.gpsimd.dma_start(wq_sb, io["wq"].rearrange("(c p) o -> p c o", p=P))
            wk_sb = wpool.tile([P, EC, E], bf16, tag="wk")
            nc.gpsimd.dma_start(wk_sb, io["wk"].rearrange("(c p) o -> p c o", p=P))
            wv_sb = wpool.tile([P, EC, E], bf16, tag="wv")
            nc.gpsimd.dma_start(wv_sb, io["wv"].rearrange("(c p) o -> p c o", p=P))
            wo_sb = wpool.tile([P, EC, E], bf16, tag="wo")
            nc.gpsimd.dma_start(wo_sb, io["wo"].rearrange("(c p) o -> p c o", p=P))
            bob = wpool.tile([P, E], f32, tag="bob")
            nc.gpsimd.dma_start(bob, _bcast_row_ap(io["gvecs"], 3))

            for g in range(ng):
                # FFN weight prefetch: issued once group-0 inputs are in
                # flight so it overlaps pass A compute instead of blocking
                # the first projections
                if g == 2:
                    nc.gpsimd.dma_start(
                        w1_sb, io["w1"].rearrange("(c p) o -> p c o", p=P)
                    )
                elif g == 3:
                    nc.gpsimd.dma_start(
                        w2_sb, io["w2"].rearrange("(c p) o -> p c o", p=P)
                    )
                rows = slice(g * TOKG, (g + 1) * TOKG)
                # token-major q (residual only)
                qb = tin_pool.tile([P, GROUP, E], bf16, tag="qb")
                nc.sync.dma_start(
                    qb, io["xq"][rows, :].rearrange("(a p) e -> p a e", p=P)
                )
                # E-major q/k/v straight from DRAM via xbar transpose
                qT = tT_pool.tile([P, EC, TOKG], bf16, tag="qT")
                nc.sync.dma_start_transpose(qT, io["xq"][rows, :])
                kT = tT_pool.tile([P, EC, TOKG], bf16, tag="kT")
                nc.sync.dma_start_transpose(kT, io["xk"][rows, :])
                vT = tT_pool.tile([P, EC, TOKG], bf16, tag="vT")
                nc.scalar.dma_start_transpose(vT, io["xv"][rows, :])

                # mask additive tile, broadcast across partitions
                maddt = madd_pool.tile([P, GROUP, P], f32, tag="madd")
                nc.gpsimd.dma_start(
                    maddt, _bcast_rows_ap(io["madd"], g * GROUP, GROUP)
                )

                # --- q/k projections, E-major [P, H, TOKG] ---
                qTb = qkt_pool.tile([P, H, TOKG], bf16, tag="qTb")
                kTb = qkt_pool.tile([P, H, TOKG], bf16, tag="kTb")
                for srcT, dst, w_sb in ((qT, qTb, wq_sb), (kT, kTb, wk_sb)):
                    for eo in range(EC):
                        ps = psA.tile([P, TOKG], f32, tag="pA")
                        for ci in range(EC):
                            nc.tensor.matmul(
                                ps,
                                lhsT=w_sb[:, ci, eo * P : (eo + 1) * P],
                                rhs=srcT[:, ci, :],
                                start=(ci == 0),
                                stop=(ci == EC - 1),
                            )
                        pcopy(dst[:, eo, :], ps)

                # --- v projection (token-major out, per pair; ci outer) ---
                v_tok = vtok_pool.tile([P, GROUP, E], bf16, tag="v_tok")
                for a in range(GROUP):
                    pv = psbig.tile([P, E], f32, tag="pbig")
                    for ci in range(EC):
                        for n0, nsz in ((0, 512), (512, 128)):
                            nc.tensor.matmul(
                                pv[:, n0 : n0 + nsz],
                                lhsT=vT[:, ci, a * P : (a + 1) * P],
                                rhs=wv_sb[:, ci, n0 : n0 + nsz],
                                start=(ci == 0),
                                stop=(ci == EC - 1),
                            )
                    pcopy(v_tok[:, a, :], pv)

                # --- attention + out-proj + residual, in 2-pair waves so
                # pair a+1's PE work hides pair a's softmax latency ---
                outT = outt_pool.tile([P, H, TOKG], bf16, tag="outT")
                mv4 = spool.tile([P, GROUP, 2], f32, tag="mv4")
                x1s = [None] * GROUP
                attn5s = [None] * GROUP

                def _energy_softmax(a):
                    tsl = slice(a * P, (a + 1) * P)
                    # energy^T[k, (h, q)] for all 5 heads
                    pse5 = psbig.tile([P, H, P], f32, tag="pbig")
                    for h in range(H):
                        nc.tensor.matmul(
                            pse5[:, h, :], lhsT=kTb[:, h, tsl], rhs=qTb[:, h, tsl],
                            start=True, stop=True,
                        )
                    # + additive mask on the q (free) axis
                    nc.vector.tensor_tensor(
                        pse5, pse5,
                        maddt[:, a, None, :].to_broadcast([P, H, P]),
                        op=ALU.add,
                    )
                    # softmax over q (free axis), scaled by 1/sqrt(128);
                    # energies are ~N(0,1) after scaling -> no max-sub needed
                    attn5 = attn_pool.tile([P, H, P], bf16, tag="asb")
                    nc.scalar.activation(attn5, pse5, ACTF.Exp, scale=SCALE)
                    ssum5 = spool.tile([P, H], f32, tag="ssum5")
                    nc.vector.tensor_reduce(ssum5, attn5, op=ALU.add, axis=AX)
                    rec5 = spool.tile([P, H], f32, tag="rec5")
                    nc.vector.reciprocal(rec5, ssum5)
                    nc.vector.tensor_tensor(
                        attn5, attn5, rec5[:, :, None].to_broadcast([P, H, P]),
                        op=ALU.mult,
                    )
                    attn5s[a] = attn5

                def _av(a):
                    tsl = slice(a * P, (a + 1) * P)
                    # out^T[d, h, q] = sum_l v[l,(h,d)] attn^T[h, l, q]
                    pso5 = psbig.tile([P, H, P], f32, tag="pbig")
                    for h in range(H):
                        nc.tensor.matmul(
                            pso5[:, h, :],
                            lhsT=v_tok[:, a, h * P : (h + 1) * P],
                            rhs=attn5s[a][:, h, :],
                            start=True, stop=True,
                        )
                    pcopy(outT[:, :, tsl], pso5)

                def _woproj(a):
                    tsl = slice(a * P, (a + 1) * P)
                    # attention_out = out @ Wo (token-major; h outer to share
                    # weights across the two PSUM column regions)
                    psw = psbig.tile([P, E], f32, tag="pbig")
                    for h in range(H):
                        for n0, nsz in ((0, 512), (512, 128)):
                            nc.tensor.matmul(
                                psw[:, n0 : n0 + nsz],
                                lhsT=outT[:, h, tsl],
                                rhs=wo_sb[:, h, n0 : n0 + nsz],
                                start=(h == 0),
                                stop=(h == H - 1),
                            )
                    # x1 = psw + bo + query (bf16 residual), stats
                    x1 = x1_pool.tile([P, E], f32, tag="x1")
                    nc.vector.tensor_tensor(x1, psw, bob, op=ALU.add)
                    nc.vector.tensor_add(x1, x1, qb[:, a, :])
                    st = spool.tile([P, 2, 6], f32, tag="bnst")
                    nc.vector.bn_stats(st[:, 0, :], x1[:, 0:320])
                    nc.vector.bn_stats(st[:, 1, :], x1[:, 320:640])
                    nc.vector.bn_aggr(mv4[:, a, :], st)
                    x1s[a] = x1

                for w in (0, 2):
                    _energy_softmax(w)
                    _energy_softmax(w + 1)
                    _av(w)
                    _av(w + 1)
                    _woproj(w)
                    _woproj(w + 1)

                # batched LN1 normalize: one ACT Sqrt per group (2 table
                # swaps per group instead of 2 per pair)
                sd4 = spool.tile([P, GROUP], f32, tag="sd4")
                nc.scalar.activation(sd4, mv4[:, :, 1], ACTF.Sqrt, bias=epst)
                rstd4 = spool.tile([P, GROUP], f32, tag="rstd4")
                nc.vector.reciprocal(rstd4, sd4)
                nmr4 = spool.tile([P, GROUP], f32, tag="nmr4")
                nc.vector.tensor_tensor(nmr4, mv4[:, :, 0], rstd4, op=ALU.mult)
                nc.vector.tensor_scalar_mul(nmr4, nmr4, -1.0)
                xnb = xnpool.tile([P, GROUP, E], bf16, tag="xnb")
                for a in range(GROUP):
                    nc.vector.tensor_scalar(
                        xnb[:, a, :], x1s[a],
                        rstd4[:, a : a + 1], nmr4[:, a : a + 1],
                        op0=ALU.mult, op1=ALU.add,
                    )
                xnb_tiles[g] = xnb

        # ================= PASS B: FFN + LN2 =================
        with ExitStack() as bctx:
            wbpool = bctx.enter_context(tc.tile_pool(name="wB", bufs=1))
            xt_pool = bctx.enter_context(tc.tile_pool(name="xT", bufs=2))
            ht_pool = bctx.enter_context(tc.tile_pool(name="hT", bufs=4))
            fft_pool = bctx.enter_context(tc.tile_pool(name="ffT", bufs=2))
            fftok_pool = bctx.enter_context(tc.tile_pool(name="fftok", bufs=2))
            x2_pool = bctx.enter_context(tc.tile_pool(name="x2", bufs=4))
            out_pool = bctx.enter_context(tc.tile_pool(name="outg", bufs=4))
            spoolb = bctx.enter_context(tc.tile_pool(name="statsb", bufs=4))
            psB = bctx.enter_context(tc.tile_pool(name="psB", bufs=2, space="PSUM"))
            psf_pool = bctx.enter_context(
                tc.tile_pool(name="psf", bufs=EC, space="PSUM")
            )

            bvec = []
            for r in range(3):  # g1, g2, be2
                t = wbpool.tile([P, E], f32, tag=f"bc{r}")
                nc.gpsimd.dma_start(t, _bcast_row_ap(io["gvecs"], r))
                bvec.append(t)
            g1b, g2b, be2b = bvec
            b1t = wbpool.tile([P, FC], f32, tag="b1t")
            nc.gpsimd.dma_start(b1t, io["b1t"])
            b2t = wbpool.tile([P, EC], f32, tag="b2t")
            nc.gpsimd.dma_start(b2t, io["b2t"])

            for g in range(ng):
                # E-major x via DMA xbar transpose: [P, (a, eo), t]
                xT = xt_pool.tile([P, GROUP, EC, P], bf16, tag="xT")
                nc.sync.dma_start_transpose(xT, xnb_tiles[g])

                # interleaved FFN: each hT chunk immediately feeds the 5
                # E-major FFN2 partial accumulators
                psf = []
                for eo in range(EC):
                    pf = psf_pool.tile([P, TOKG], f32, tag="psf")
                    psf.append(pf)
                for f in range(FC):
                    ps = psB.tile([P, TOKG], f32, tag="pB")
                    for ci in range(EC):
                        nc.tensor.matmul(
                            ps,
                            lhsT=w1_sb[:, ci, f * P : (f + 1) * P],
                            rhs=xT[:, :, ci, :],
                            start=(ci == 0),
                            stop=(ci == EC - 1),
                        )
                    ht = ht_pool.tile([P, TOKG], bf16, tag="ht")
                    if f % 4 == 3:
                        # DVE takes every 4th chunk; ACT (which has slack)
                        # takes the rest so DVE stays free for LN2 chains
                        nc.vector.tensor_scalar(
                            ht, ps, b1t[:, f : f + 1], 0.0,
                            op0=ALU.add, op1=ALU.max,
                        )
                    else:
                        nc.scalar.activation(
                            ht, ps, ACTF.Relu, bias=b1t[:, f : f + 1], scale=1.0
                        )
                    for eo in range(EC):
                        nc.tensor.matmul(
                            psf[eo],
                            lhsT=w2_sb[:, f, eo * P : (eo + 1) * P],
                            rhs=ht,
                            start=(f == 0),
                            stop=(f == FC - 1),
                        )

                # ff^T + (b2 + be1) -> bf16, [P, (a, eo), t] layout
                ffT = fft_pool.tile([P, GROUP, EC, P], bf16, tag="ffT")
                for eo in range(EC):
                    src = psf[eo].rearrange("p (a t) -> p a t", a=GROUP)
                    # all on ACT: frees psf slots for the next group's FFN2
                    # without waiting on DVE (busy with LN2 chains)
                    nc.scalar.activation(
                        ffT[:, :, eo, :], src, ACTF.Identity,
                        bias=b2t[:, eo : eo + 1],
                    )

                # back to token-major per pair
                ff_tok = fftok_pool.tile([P, GROUP, E], bf16, tag="fftok")
                for a in range(GROUP):
                    # out must stay 3D [P, EC, P]: a 2D [P, 640] view makes
                    # dma_start_transpose mis-tile the xbar output
                    nc.scalar.dma_start_transpose(
                        ff_tok[:, a].rearrange("p (c t) -> p c t", c=EC),
                        ffT[:, a],
                    )

                # x2 = xnb * g1 + ff (be1 + b2 already in ff), then LN2
                for a in range(GROUP):
                    x2 = x2_pool.tile([P, E], f32, tag="x2")
                    nc.gpsimd.tensor_tensor(
                        x2, xnb_tiles[g][:, a, :], g1b, op=ALU.mult
                    )
                    nc.vector.tensor_add(x2, x2, ff_tok[:, a, :])
                    st = spoolb.tile([P, 2, 6], f32, tag="bnstb")
                    nc.vector.bn_stats(st[:, 0, :], x2[:, 0:320])
                    nc.vector.bn_stats(st[:, 1, :], x2[:, 320:640])
                    mv = spoolb.tile([P, 2], f32, tag="mvb")
                    nc.vector.bn_aggr(mv, st)
                    sd = spoolb.tile([P, 1], f32, tag="sdb")
                    nc.scalar.activation(sd, mv[:, 1:2], ACTF.Sqrt, bias=epst)
                    rstd = spoolb.tile([P, 1], f32, tag="rstdb")
                    nc.vector.reciprocal(rstd, sd)
                    nmr = spoolb.tile([P, 1], f32, tag="nmrb")
                    nc.vector.tensor_scalar(
                        nmr, mv[:, 0:1], rstd, -1.0, op0=ALU.mult, op1=ALU.mult
                    )
                    xo = x2_pool.tile([P, E], f32, tag="xo")
                    nc.vector.tensor_scalar(
                        xo, x2, rstd, nmr, op0=ALU.mult, op1=ALU.add
                    )
                    # affine split across Pool/DVE, alternating per pair;
                    # store per pair so the tail drains promptly
                    t2 = x2_pool.tile([P, E], f32, tag="t2")
                    outp = out_pool.tile([P, E], f32, tag="outp")
                    if a % 2:
                        nc.gpsimd.tensor_tensor(t2, xo, g2b, op=ALU.mult)
                        nc.vector.tensor_add(outp, t2, be2b)
                    else:
                        nc.vector.tensor_tensor(t2, xo, g2b, op=ALU.mult)
                        nc.gpsimd.tensor_tensor(outp, t2, be2b, op=ALU.add)
                    nc.sync.dma_start(
                        io["out"][
                            g * TOKG + a * P : g * TOKG + (a + 1) * P, :
                        ],
                        outp,
                    )


@functools.lru_cache(maxsize=8)
def _build(npairs=NPAIRS, repeat=1, loop=1):
    nc = bacc.Bacc(
        "TRN2", target_bir_lowering=False, debug=False, num_devices=NCORES
    )
    io = {
        "xq": nc.dram_tensor("xq", [npairs * P, E], bf16, kind="ExternalInput").ap(),
        "xk": nc.dram_tensor("xk", [npairs * P, E], bf16, kind="ExternalInput").ap(),
        "xv": nc.dram_tensor("xv", [npairs * P, E], bf16, kind="ExternalInput").ap(),
        "madd": nc.dram_tensor("madd", [npairs, P], f32, kind="ExternalInput").ap(),
        "wq": nc.dram_tensor("wq", [E, E], bf16, kind="ExternalInput").ap(),
        "wk": nc.dram_tensor("wk", [E, E], bf16, kind="ExternalInput").ap(),
        "wv": nc.dram_tensor("wv", [E, E], bf16, kind="ExternalInput").ap(),
        "wo": nc.dram_tensor("wo", [E, E], bf16, kind="ExternalInput").ap(),
        "w1": nc.dram_tensor("w1", [E, F], bf16, kind="ExternalInput").ap(),
        "w2": nc.dram_tensor("w2", [F, E], bf16, kind="ExternalInput").ap(),
        "b1t": nc.dram_tensor("b1t", [P, FC], f32, kind="ExternalInput").ap(),
        "b2t": nc.dram_tensor("b2t", [P, EC], f32, kind="ExternalInput").ap(),
        "gvecs": nc.dram_tensor("gvecs", [4, E], f32, kind="ExternalInput").ap(),
        "out": nc.dram_tensor("out", [npairs * P, E], f32, kind="ExternalOutput").ap(),
    }
    with tile.TileContext(nc) as tc:
        if loop > 1:
            # hardware loop: R full iterations inside one launch (for timing)
            with tc.For_i(0, loop, 1):
                for _ in range(repeat):
                    _emit(tc, io, npairs)
        else:
            for _ in range(repeat):
                _emit(tc, io, npairs)
    nc.compile()
    return nc


def _prep_in_maps(value, key, query, mask, Wv, Wk, Wq, Wo, bo, W1, b1, W2, b2,
                  g1, be1, g2, be2):
    bfl = ml_dtypes.bfloat16
    w1p = (np.asarray(g1, np.float32)[:, None] * np.asarray(W1, np.float32))
    b1p = np.asarray(b1, np.float32) + np.asarray(be1, np.float32) @ np.asarray(
        W1, np.float32
    )
    b2p = np.asarray(b2, np.float32) + np.asarray(be1, np.float32)
    shared = {
        "wq": np.ascontiguousarray(Wq.astype(bfl)),
        "wk": np.ascontiguousarray(Wk.astype(bfl)),
        "wv": np.ascontiguousarray(Wv.astype(bfl)),
        "wo": np.ascontiguousarray(Wo.astype(bfl)),
        "w1": np.ascontiguousarray(w1p.astype(bfl)),
        "w2": np.ascontiguousarray(W2.astype(bfl)),
        "b1t": np.ascontiguousarray(b1p.reshape(FC, P).T.astype(np.float32)),
        "b2t": np.ascontiguousarray(b2p.reshape(EC, P).T.astype(np.float32)),
        "gvecs": np.ascontiguousarray(
            np.stack([g1, g2, be2, bo]).astype(np.float32)
        ),
    }
    in_maps = []
    npc = 64 // NCORES  # n-values per core
    for c in range(NCORES):
        nsl = slice(c * npc, (c + 1) * npc)
        madd = np.where(
            mask[nsl, :, :, 0] == 0, np.float32(-1e20), np.float32(0.0)
        ).reshape(NPAIRS, P).astype(np.float32)
        in_maps.append(
            {
                "xq": np.ascontiguousarray(
                    query[nsl].reshape(NPAIRS * P, E).astype(bfl)
                ),
                "xk": np.ascontiguousarray(
                    key[nsl].reshape(NPAIRS * P, E).astype(bfl)
                ),
                "xv": np.ascontiguousarray(
                    value[nsl].reshape(NPAIRS * P, E).astype(bfl)
                ),
                "madd": np.ascontiguousarray(madd),
                **shared,
            }
        )
    return in_maps


def kernel(**inputs) -> np.ndarray:
    nc = _build()
    in_maps = _prep_in_maps(**{
        k: np.asarray(v) for k, v in inputs.items()
    })
    res = run_bass_kernel_spmd(nc, in_maps, core_ids=list(range(NCORES)))
    out = np.concatenate([r["out"] for r in res.results], axis=0)
    return out.reshape(64, 2, P, E).astype(np.float32)


def run_traced(**inputs):
    """Like kernel(), but also returns BassKernelResults with trace info."""
    nc = _build()
    in_maps = _prep_in_maps(**{k: np.asarray(v) for k, v in inputs.items()})
    res = run_bass_kernel_spmd(
        nc, in_maps, core_ids=list(range(NCORES)), trace=True
    )
    out = np.concatenate([r["out"] for r in res.results], axis=0)
    return out.reshape(64, 2, P, E).astype(np.float32), res
